# revision 40
# baseline (speedup 1.0000x reference)
"""ColBERT scoring kernel for Trainium2 (Bass/Tile), data-parallel over batch.

Reference computation (per batch b):
    Q = l2norm(q_hidden[b] @ W)                     # [LQ, DIM]
    D = l2norm((d_hidden[b] * mask[b,:,None]) @ W)  # [LD, DIM]
    score[b] = sum_q max_k (Q @ D.T)[q, k]

Sharding: batch dim B=64 split over 8 NeuronCores (8 batches/core), W replicated.

Device layout: everything stays transposed (DIM/H on partitions) so all matmuls
contract over the partition dim with moving free dims of 512:
  P_d^T[DIM, LD] = W_chunk^T @ d^T_chunk            (accumulate 6 chunks of H)
  doc norms: colsums of (P_d^T)^2 via an all-ones [128,128] stationary matmul,
  which lands the per-column sumsq already broadcast across all 128 partitions;
  1/sqrt via one ScalarE Abs_reciprocal_sqrt op (DVE reciprocal is an 8
  cycle/elem iterative op - 3us per [128,512] tile - so it is avoided on the
  broadcast tile).
  sim[LQ, LD] = (Q^T)^T @ D^T, reduce_max over the free dim; the query-side
  norm commutes with max over k, so it is applied after the reduce (its
  reciprocal runs on a compact [128,8] tile where it is cheap).
  Final sum over LQ via a ones-column matmul -> [8] scores.

Mask folds into d_hidden on the host (linear). Host pre-packs shards so each
DMA is one fully-contiguous-per-partition read.

COLBERT_MM_MODE:
  f16 (default): inputs cast to fp16 on host (same 10-bit mantissa as TF32 but
       half the HBM traffic; PSUM accumulation is fp32).
  f32r: TF32 matmul operands (host pre-rounds), fp16 post-projection operands.
  f32 : exact fp32 everywhere (PE runs 4 cycles/row).
COLBERT_RSQRT=act (default) | exact  (Sqrt + DVE reciprocal fallback)
"""

import os

import numpy as np

B, LQ, LD, H, DIM = 64, 128, 512, 768, 128
NCORES = 8
BLOC = B // NCORES  # 8 batches per core
P = 128
HC = H // P  # 6 contraction chunks
NQ = BLOC * LQ  # 1024
EPS2 = 1e-24  # eps^2 so that sqrt(ss + eps^2) ~ max(norm, 1e-12)

MM_MODE = os.environ.get("COLBERT_MM_MODE", "f16")
RSQRT = os.environ.get("COLBERT_RSQRT", "act")

_cache = {}


def _round_tf32(a):
    """Round fp32 array to TF32 (10-bit mantissa, RNE) in an fp32 container."""
    b = np.ascontiguousarray(a, dtype=np.float32).view(np.uint32)
    bias = np.uint32(0x00000FFF) + ((b >> np.uint32(13)) & np.uint32(1))
    b = (b + bias) & np.uint32(0xFFFFE000)
    return b.view(np.float32)


def _build(kd=LD, clamp0=False):
    import concourse.bass as bass
    import concourse.tile as tile
    from concourse import bacc, mybir

    f32 = mybir.dt.float32
    f16 = mybir.dt.float16
    bf16 = mybir.dt.bfloat16
    if MM_MODE == "bf16":
        in_dt = bf16
    elif MM_MODE == "f16":
        in_dt = f16
    elif MM_MODE == "f32r":
        in_dt = mybir.dt.float32r
    else:
        in_dt = f32
    if MM_MODE == "bf16":
        mid_dt = bf16
    elif MM_MODE in ("f16", "f32r"):
        mid_dt = f16
    else:
        mid_dt = f32

    sq_dt = mid_dt

    nc = bacc.Bacc("TRN2", target_bir_lowering=False, debug=False,
                   num_devices=NCORES)

    qt = nc.dram_tensor("qt", [P, HC * NQ], in_dt, kind="ExternalInput").ap()
    dt = nc.dram_tensor("dt", [BLOC, P, HC * kd], in_dt, kind="ExternalInput").ap()
    wt = nc.dram_tensor("wt", [P, HC * DIM], in_dt, kind="ExternalInput").ap()
    out = nc.dram_tensor("scores", [BLOC, 1], f32, kind="ExternalOutput").ap()

    with tile.TileContext(nc) as tc:
        with (
            tc.tile_pool(name="const", bufs=1) as const,
            tc.tile_pool(name="dload", bufs=BLOC) as dload,
            tc.tile_pool(name="work", bufs=3) as work,
            tc.tile_pool(name="ps_misc", bufs=1, space="PSUM") as ps_misc,
            tc.tile_pool(name="ps_d", bufs=4, space="PSUM") as ps_d,
            tc.tile_pool(name="ps_ssb", bufs=1, space="PSUM") as ps_ssb,
            tc.tile_pool(name="ps_sim", bufs=2, space="PSUM") as ps_sim,
        ):
            # ---- constants ----
            w_sb = const.tile([P, HC * DIM], in_dt)
            nc.scalar.dma_start(out=w_sb, in_=wt)
            ones_pk = const.tile([P, P], sq_dt)
            nc.vector.memset(ones_pk, 1.0)
            ones_c1 = const.tile([P, 1], sq_dt)
            nc.vector.memset(ones_c1, 1.0)
            ones_f32 = const.tile([P, 1], f32)
            nc.vector.memset(ones_f32, 1.0)
            eps_c = const.tile([P, 1], f32)
            nc.vector.memset(eps_c, EPS2)

            qT_all = const.tile([P, NQ], mid_dt)      # Q^T for all batches
            rq = const.tile([P, BLOC], f32)           # 1/||q|| per (LQ, b)
            scores_cols = const.tile([P, BLOC], f32)  # per-q maxsim, col per b

            # ---- all input DMAs issue up front (HBM is the roofline) ----
            q_all = const.tile([P, HC * NQ], in_dt)
            qv = q_all.rearrange("p (c n) -> p c n", c=HC)
            qtv = qt.rearrange("p (c n) -> p c n", c=HC)
            dt_tiles = []
            for b in range(BLOC):
                dt_t = dload.tile([P, HC * kd], in_dt, name=f"dt_t{b}",
                                  tag="dt_t")
                nc.scalar.dma_start(out=dt_t, in_=dt[b])
                dt_tiles.append(dt_t)
                if b in (1, 3):  # the half each Q-proj group reads, just in
                    g = b // 2   # time for where the PE stream reaches it
                    nc.scalar.dma_start(out=qv[:, :, g * 512:(g + 1) * 512],
                                        in_=qtv[:, :, g * 512:(g + 1) * 512])

            sqq = work.tile([P, NQ], sq_dt, tag="sqq", bufs=1)
            pssq = ps_misc.tile([P, BLOC], f32, tag="misc")

            def q_proj_g(g):
                psq = ps_d.tile([P, 512], f32, name=f"psq{g}", tag="pd")
                for c in range(HC):
                    nc.tensor.matmul(
                        psq,
                        w_sb[:, c * DIM:(c + 1) * DIM],
                        q_all[:, c * NQ + g * 512:c * NQ + (g + 1) * 512],
                        start=(c == 0), stop=(c == HC - 1),
                    )
                nc.vector.tensor_copy(qT_all[:, g * 512:(g + 1) * 512], psq)
                nc.scalar.square(sqq[:, g * 512:(g + 1) * 512], psq)
                for b in range(4 * g, 4 * g + 4):
                    nc.tensor.matmul(
                        pssq[:, b:b + 1],
                        sqq[:, b * LQ:(b + 1) * LQ],
                        ones_c1,
                        start=True, stop=True,
                    )
                if g == 1:
                    nc.scalar.activation(
                        rq, pssq,
                        mybir.ActivationFunctionType.Abs_reciprocal_sqrt,
                        bias=eps_c)

            def post_stage(b, pd):
                # doc norms, broadcast across partitions via ones matmul
                sqd = work.tile([P, kd], sq_dt, tag="sqd", name=f"sqd{b}")
                nc.scalar.square(sqd, pd)
                pssb = ps_ssb.tile([P, kd], f32, name=f"pssb{b}", tag="pssb")
                nc.tensor.matmul(pssb, ones_pk, sqd, start=True, stop=True)
                r = work.tile([P, kd], f32, tag="r", name=f"r{b}")
                nc.scalar.activation(
                    r, pssb,
                    mybir.ActivationFunctionType.Abs_reciprocal_sqrt,
                    bias=eps_c)
                dhat = work.tile([P, kd], mid_dt, tag="dhat", name=f"dhat{b}")
                nc.vector.tensor_mul(dhat, pd, r)
                # late interaction
                psim = ps_sim.tile([P, kd], f32, name=f"psim{b}", tag="psim")
                nc.tensor.matmul(
                    psim,
                    qT_all[:, b * LQ:(b + 1) * LQ],
                    dhat,
                    start=True, stop=True,
                )
                m = work.tile([P, 1], f32, tag="m", name=f"m{b}", bufs=4)
                nc.vector.reduce_max(m, psim, axis=mybir.AxisListType.X)
                if clamp0:
                    nc.vector.tensor_scalar(
                        out=scores_cols[:, b:b + 1], in0=m, scalar1=0.0,
                        scalar2=rq[:, b:b + 1], op0=mybir.AluOpType.max,
                        op1=mybir.AluOpType.mult)
                else:
                    nc.vector.tensor_mul(scores_cols[:, b:b + 1], m,
                                         rq[:, b:b + 1])

            # ---- D loop: q-proj groups interleave with the first rounds;
            # pairs share weight loads; last two batches run singly so their
            # projections start the moment their data lands
            def proj(b, pd):
                for c in range(HC):
                    nc.tensor.matmul(
                        pd,
                        w_sb[:, c * DIM:(c + 1) * DIM],
                        dt_tiles[b][:, c * kd:(c + 1) * kd],
                        start=(c == 0), stop=(c == HC - 1),
                    )

            for rnd in range(3):
                b0, b1 = 2 * rnd, 2 * rnd + 1
                pd0 = ps_d.tile([P, kd], f32, name=f"pd{b0}", tag="pd")
                pd1 = ps_d.tile([P, kd], f32, name=f"pd{b1}", tag="pd")
                for c in range(HC):
                    wc = w_sb[:, c * DIM:(c + 1) * DIM]
                    nc.tensor.matmul(pd0, wc,
                                     dt_tiles[b0][:, c * kd:(c + 1) * kd],
                                     start=(c == 0), stop=(c == HC - 1))
                    nc.tensor.matmul(pd1, wc,
                                     dt_tiles[b1][:, c * kd:(c + 1) * kd],
                                     start=(c == 0), stop=(c == HC - 1))
                if rnd == 0:
                    q_proj_g(0)
                    q_proj_g(1)
                post_stage(b0, pd0)
                post_stage(b1, pd1)
            for b in (6, 7):
                pd = ps_d.tile([P, kd], f32, name=f"pd{b}", tag="pd")
                proj(b, pd)
                post_stage(b, pd)

            # ---- final: per-batch sum over LQ ----
            pfin = ps_misc.tile([BLOC, 1], f32, tag="misc")
            nc.tensor.matmul(pfin, scores_cols, ones_f32, start=True, stop=True)
            scores_sb = work.tile([BLOC, 1], f32, tag="fin", bufs=1)
            nc.vector.tensor_copy(scores_sb, pfin)
            nc.sync.dma_start(out=out, in_=scores_sb)

    nc.compile()
    return nc


K_CAP = 352  # 8.5 sigma above Binomial(512, 0.5) mean; overflow -> full path


def kernel(q_hidden, d_hidden, W, doc_mask):
    from concourse.bass_utils import run_bass_kernel_spmd

    q_hidden = np.asarray(q_hidden, dtype=np.float32)
    d_hidden = np.asarray(d_hidden, dtype=np.float32)
    W = np.asarray(W, dtype=np.float32)
    doc_mask = np.asarray(doc_mask)

    counts = (np.asarray(doc_mask) != 0).sum(axis=1)
    # compaction drops the mask multiply, which is only valid for 0/1 masks
    compact = counts.max() <= K_CAP and bool(np.isin(doc_mask, (0, 1)).all())
    kd = K_CAP if compact else LD
    key = ("nc", kd, compact)
    if key not in _cache:
        _cache[key] = _build(kd=kd, clamp0=compact)
    nc = _cache[key]

    if MM_MODE == "bf16":
        import ml_dtypes
        cvt = lambda a: np.ascontiguousarray(a.astype(ml_dtypes.bfloat16))
    elif MM_MODE == "f16":
        cvt = lambda a: np.ascontiguousarray(a, dtype=np.float16)
    elif MM_MODE == "f32r":
        cvt = _round_tf32
    else:
        cvt = lambda a: np.ascontiguousarray(a, dtype=np.float32)

    # host-side shard prep (pack so every DMA line is contiguous per partition)
    if compact:
        # keep only unmasked doc tokens (zero rows only reach the score via
        # max(.,0), applied on-device); pad to K_CAP with zero rows
        d_m = np.zeros((B, K_CAP, H), dtype=np.float32)
        mask_b = np.asarray(doc_mask) != 0
        for b in range(B):
            sel = d_hidden[b][mask_b[b]]
            d_m[b, :len(sel)] = sel
    else:
        d_m = d_hidden * doc_mask[..., None].astype(np.float32)
    # wt[p, c*DIM + j] = W[c*P + p, j]
    wt = cvt(W.reshape(HC, P, DIM).transpose(1, 0, 2).reshape(P, HC * DIM))
    in_maps = []
    for c in range(NCORES):
        sl = slice(c * BLOC, (c + 1) * BLOC)
        # qt[p, c*NQ + b*LQ + l] = q[b, l, c*P + p]
        qt = cvt(q_hidden[sl].reshape(BLOC, LQ, HC, P)
                 .transpose(3, 2, 0, 1).reshape(P, HC * NQ))
        # dt[b, p, c*kd + l] = d_m[b, l, c*P + p]
        dtc = cvt(d_m[sl].reshape(BLOC, kd, HC, P)
                  .transpose(0, 3, 2, 1).reshape(BLOC, P, HC * kd))
        in_maps.append({"qt": qt, "dt": dtc, "wt": wt})

    trace = os.environ.get("COLBERT_TRACE", "0") == "1"
    res = run_bass_kernel_spmd(nc, in_maps, core_ids=list(range(NCORES)),
                               trace=trace)
    _cache["last_results"] = res
    return np.concatenate([r["scores"].reshape(BLOC) for r in res.results])
